# revision 2
# baseline (speedup 1.0000x reference)
"""TRN2 Bass/Tile kernel v2: 16-head MHA (N=2, S=2048, D=1024) on 8 NeuronCores.

Sharding (hardcoded): core c = 4*n + g runs batch n (data parallel) and head
group g (tensor parallel, 4 heads).  Wq/Wk/Wv column-sharded [1024, 256], Wp
row-sharded [256, 1024].  Host sums the 4 partials per batch and adds
(bv @ Wp + bp).

v2 changes vs baseline:
  - all inputs host-cast to bf16 (matmuls were bf16 anyway): halves DMA-in,
    removes the on-chip fp32->bf16 cast traffic on DVE/ACT entirely
  - scores matmuls row-tiled: the two heads of a pair occupy PE row strips
    0-63 / 64-127 (K=64 contraction each), so their LDWEIGHTS/streams overlap
    (measured 460 -> 201 ns/MM)
  - softmax exp runs on ScalarE at [128, 1024] tiles; the whole attention
    phase is paced to keep ACT (the roofline engine: 16.8M exp elems/core
    ~ 128 us) continuously busy
  - V projection is emitted just-in-time inside the first attention block's
    key-chunk loop (xv DMA is still in flight then)
  - output projection is split per query block and interleaved into the next
    block's attention; partial outs are stored bf16
"""

import numpy as np

N, S, D = 2, 2048, 1024
H, HD = 16, 64
NHL = 4                 # heads per core
DH = NHL * HD           # 256 local channels
P = 128
KC = D // P             # 8 contraction chunks for the projections
SC = S // P             # 16 key chunks
IH = 1024               # query block width
NI = S // IH            # 2 query blocks

_built = {}


def _emit(tc, out, xqt, xkt, xvt, wq, wk, wv, wp, bq, bk, stage="full"):
    from concourse import mybir

    nc = tc.nc
    f32 = mybir.dt.float32
    bf16 = mybir.dt.bfloat16
    Exp = mybir.ActivationFunctionType.Exp
    MUL = mybir.AluOpType.mult
    ADD = mybir.AluOpType.add

    with (
        tc.tile_pool(name="const", bufs=1) as cpool,
        tc.tile_pool(name="work", bufs=1) as wpool,
        tc.tile_pool(name="e", bufs=5) as epool,
        tc.tile_pool(name="small", bufs=2) as spool,
        tc.tile_pool(name="ob", bufs=4) as opool,
        tc.tile_pool(name="ps", bufs=1, space="PSUM") as ps,
    ):
        # ---------- SBUF tiles ----------
        wq_sb = cpool.tile([P, KC, DH], bf16)
        wk_sb = cpool.tile([P, KC, DH], bf16)
        wv_sb = cpool.tile([P, KC, DH], bf16)
        wp_sb = cpool.tile([P, 2, D], bf16)
        bq_sb = cpool.tile([P, 2], f32)
        bk_sb = cpool.tile([P, 2], f32)
        xq_sb = wpool.tile([P, KC, S], bf16)
        xk_sb = wpool.tile([P, KC, S], bf16)
        xv_sb = wpool.tile([P, KC, S], bf16)
        qt_sb = wpool.tile([P, 2, S], bf16)
        kt_sb = wpool.tile([P, 2, S], bf16)
        v_sb = wpool.tile([P, SC, NHL, HD + 1], bf16)
        ot_sb = {}  # (pair, i) -> [128, IH] bf16

        # ACT exp-table preload: tiny exp early so the ~2.7us table DMA
        # overlaps the input loads.
        warm = spool.tile([1, 8], f32, tag="warm", name="warm")
        nc.vector.memset(warm[:], 0.0)
        wout = spool.tile([1, 8], bf16, tag="wout", name="wout")
        nc.scalar.activation(wout[:], warm[:], Exp, scale=0.125)

        nc.vector.memset(v_sb[:], 1.0)

        # ---------- DMA issue ----------
        # weights + biases on the SWDGE (gpsimd) queue, activations on the
        # HWDGE (sync) queue; the two rings drain concurrently.
        nc.gpsimd.dma_start(wq_sb[:], wq.rearrange("(kc p) d -> p kc d", p=P))
        nc.gpsimd.dma_start(wk_sb[:], wk.rearrange("(kc p) d -> p kc d", p=P))
        nc.gpsimd.dma_start(bq_sb[:], bq.rearrange("(c p) -> p c", p=P))
        nc.gpsimd.dma_start(bk_sb[:], bk.rearrange("(c p) -> p c", p=P))
        nc.gpsimd.dma_start(wv_sb[:], wv.rearrange("(kc p) d -> p kc d", p=P))
        nc.gpsimd.dma_start(wp_sb[:], wp.rearrange("(c p) e -> p c e", p=P))
        for kc in range(KC):
            nc.sync.dma_start(xq_sb[:, kc, :], xqt[kc * P:(kc + 1) * P, :])
        for kc in range(KC):
            nc.sync.dma_start(xk_sb[:, kc, :], xkt[kc * P:(kc + 1) * P, :])
        for kc in range(KC):
            nc.sync.dma_start(xv_sb[:, kc, :], xvt[kc * P:(kc + 1) * P, :])

        if stage == "load":
            return

        # ---------- Q/K projections ----------
        # Q^T / K^T [256, 2048]: partitions = head-pair dims (pair c on
        # kt_sb[:, c, :], head e of the pair on partitions 64e..64e+63).
        PTAGS = ["sc0", "sc1", "av0", "av1"]

        def qkproj(x_sb, w_sb, b_sb, dst, lo, hi):
            # project x columns [lo, hi) (queries or keys)
            nblk = (hi - lo) // 512
            for c in range(2):
                pts = [
                    ps.tile([P, 512], f32, tag=PTAGS[b % 4], name=f"qk{c}_{b}")
                    for b in range(nblk)
                ]
                for kc in range(KC):
                    for b in range(nblk):
                        nc.tensor.matmul(
                            pts[b][:],
                            lhsT=w_sb[:, kc, c * P:(c + 1) * P],
                            rhs=x_sb[:, kc, lo + b * 512:lo + (b + 1) * 512],
                            start=(kc == 0),
                            stop=(kc == KC - 1),
                        )
                for b in range(nblk):
                    nc.vector.tensor_scalar(
                        dst[:, c, lo + b * 512:lo + (b + 1) * 512],
                        pts[b][:], b_sb[:, c:c + 1], None, ADD,
                    )

        def kproj_all():
            # c-wave of 4 psum groups accumulates as each xk chunk's DMA
            # lands (keeps PE paced with the DMA and HAM warm); c=0 (pair 0)
            # finishes first so attention can start while c=1 drains
            for c in range(2):
                pts = [
                    ps.tile([P, 512], f32, tag=PTAGS[b], name=f"k{c}_{b}")
                    for b in range(4)
                ]
                for kc in range(KC):
                    for b in range(4):
                        nc.tensor.matmul(
                            pts[b][:],
                            lhsT=wk_sb[:, kc, c * P:(c + 1) * P],
                            rhs=xk_sb[:, kc, b * 512:(b + 1) * 512],
                            start=(kc == 0),
                            stop=(kc == KC - 1),
                        )
                    nc.tensor.ldweights(wk_sb[:, kc, 0:P])
                for b in range(4):
                    nc.vector.tensor_scalar(
                        kt_sb[:, c, b * 512:(b + 1) * 512],
                        pts[b][:], bk_sb[:, c:c + 1], None, ADD,
                    )

        qkproj(xq_sb, wq_sb, bq_sb, qt_sb, 0, 1024)      # Q^T block i0
        qkproj(xq_sb, wq_sb, bq_sb, qt_sb, 1024, 2048)   # Q^T block i1
        kproj_all()                                      # K^T all keys

        def vproj(jc):
            pv = ps.tile([P, DH], f32, tag=PTAGS[jc % 4], name="vp")
            for kc in range(KC):
                nc.tensor.matmul(
                    pv[:],
                    lhsT=xv_sb[:, kc, jc * P:(jc + 1) * P],
                    rhs=wv_sb[:, kc, :],
                    start=(kc == 0),
                    stop=(kc == KC - 1),
                )
            nc.vector.tensor_copy(
                v_sb[:, jc, :, 0:HD], pv.rearrange("p (h d) -> p h d", d=HD)
            )

        for jc in range(SC):
            vproj(jc)

        if stage == "proj":
            return

        # ---------- attention + interleaved outproj ----------
        # out partial rows for query block i: psum <- ot_p0 @ wp[p0] + ot_p1
        # @ wp[p1]; evict bf16 and store.  Emitted as a generator so groups
        # can be interleaved into the next block's key-chunk loop.
        def outproj_groups(i):
            for qc in range(IH // P):
                r0 = i * IH + qc * P
                for eh in range(2):
                    def grp(qc=qc, eh=eh, r0=r0):
                        po = ps.tile([P, 512], f32, tag=PTAGS[(qc + eh) % 2],
                                     name="po")
                        for pr in range(2):
                            nc.tensor.matmul(
                                po[:],
                                lhsT=ot_sb[(pr, i)][:, qc * P:(qc + 1) * P],
                                rhs=wp_sb[:, pr, eh * 512:(eh + 1) * 512],
                                start=(pr == 0),
                                stop=(pr == 1),
                            )
                        ob = opool.tile([P, 512], out.dtype, tag="ob", name="ob")
                        nc.vector.tensor_copy(ob[:], po[:])
                        nc.sync.dma_start(
                            out[r0:r0 + P, eh * 512:(eh + 1) * 512], ob[:])
                    yield grp

        pending_out = None  # generator of outproj groups to interleave

        for bi, (p, i) in enumerate([(0, 0), (1, 0), (0, 1), (1, 1)]):
            av = [
                ps.tile([HD + 1, IH], f32, tag=f"av{e}", name=f"av{e}")
                for e in range(2)
            ]
            i0 = i * IH

            def emit_av(jc, ets):
                # A@V one chunk behind the scores, so the in-order PE stream
                # never waits on ScalarE exp
                for e in range(2):
                    for iq in range(2):
                        nc.tensor.matmul(
                            av[e][:, iq * 512:(iq + 1) * 512],
                            lhsT=v_sb[:, jc, 2 * p + e, :],
                            rhs=ets[e][:, iq * 512:(iq + 1) * 512],
                            start=(jc == 0),
                            stop=(jc == SC - 1),
                        )

            pending_av = None
            for jc in range(SC):
                sc = [
                    ps.tile([P, IH], f32, tag=f"sc{e}", name=f"sc{e}")
                    for e in range(2)
                ]
                for iq in range(2):
                    for e in range(2):
                        nc.tensor.matmul(
                            sc[e][:, iq * 512:(iq + 1) * 512],
                            lhsT=kt_sb[HD * e:HD * (e + 1), p,
                                       jc * P:(jc + 1) * P],
                            rhs=qt_sb[HD * e:HD * (e + 1), p,
                                      i0 + iq * 512:i0 + (iq + 1) * 512],
                            start=True,
                            stop=True,
                        )
                ets = []
                for e in range(2):
                    et = epool.tile([P, IH], bf16, tag="e", name="et")
                    nc.scalar.activation(et[:], sc[e][:], Exp, scale=0.125)
                    ets.append(et)
                if pending_av is not None:
                    emit_av(*pending_av)
                    if pending_out is not None:
                        try:
                            next(pending_out)()
                        except StopIteration:
                            pending_out = None
                    else:
                        # HAM keep-alive: pad the ACT-bound window so the PE
                        # activity monitor never sees an idle SHORT window
                        nc.tensor.ldweights(kt_sb[:, p, jc * P:jc * P + P])
                        nc.tensor.ldweights(qt_sb[:, p, i0:i0 + P])
                pending_av = (jc, ets)
            if pending_av is not None:
                emit_av(*pending_av)
            # normalize: O^T = O^T_un * (1/denom); denom is av row 64
            ot = wpool.tile([P, IH], bf16, tag=f"ot{p}_{i}", name=f"ot{p}_{i}")
            ot_sb[(p, i)] = ot
            for e in range(2):
                rec1 = spool.tile([HD + 1, IH], f32, tag="rec1", name="rec1")
                nc.vector.reciprocal(rec1[HD:HD + 1, :], av[e][HD:HD + 1, :])
                rec = spool.tile([HD, IH], f32, tag="rec", name="rec")
                nc.sync.dma_start(
                    rec[:],
                    rec1[HD:HD + 1, None, :].to_broadcast((1, HD, IH)),
                )
                if e == 0:
                    nc.vector.tensor_tensor(
                        ot[0:HD, :], av[e][0:HD, :], rec[:], MUL)
                else:
                    otmp = spool.tile([HD, IH], bf16, tag="otmp", name="otmp")
                    nc.vector.tensor_tensor(otmp[:], av[e][0:HD, :], rec[:], MUL)
                    nc.sync.dma_start(ot[HD:P, :], otmp[:])
            if stage == "attn":
                continue
            if p == 1:  # both pairs of block i done -> queue outproj
                # drain any leftover groups of the previous queue first
                if pending_out is not None:
                    for grp in pending_out:
                        grp()
                pending_out = outproj_groups(i)
        if stage == "attn":
            return
        if pending_out is not None:
            for grp in pending_out:
                grp()


def _build(reps=1, stage="full"):
    key = ("nc", reps, stage)
    if key in _built:
        return _built[key]
    import concourse.tile as tile
    from concourse import bacc, mybir

    f32 = mybir.dt.float32
    bf16 = mybir.dt.bfloat16
    nc = bacc.Bacc(
        "TRN2",
        target_bir_lowering=False,
        debug=False,
        num_devices=8,
    )
    xqt = nc.dram_tensor("xqt", [D, S], bf16, kind="ExternalInput").ap()
    xkt = nc.dram_tensor("xkt", [D, S], bf16, kind="ExternalInput").ap()
    xvt = nc.dram_tensor("xvt", [D, S], bf16, kind="ExternalInput").ap()
    wq = nc.dram_tensor("wq", [D, DH], bf16, kind="ExternalInput").ap()
    wk = nc.dram_tensor("wk", [D, DH], bf16, kind="ExternalInput").ap()
    wv = nc.dram_tensor("wv", [D, DH], bf16, kind="ExternalInput").ap()
    wp = nc.dram_tensor("wp", [DH, D], bf16, kind="ExternalInput").ap()
    bq = nc.dram_tensor("bq", [DH], f32, kind="ExternalInput").ap()
    bk = nc.dram_tensor("bk", [DH], f32, kind="ExternalInput").ap()
    out = nc.dram_tensor("out", [S, D], bf16, kind="ExternalOutput").ap()

    with tile.TileContext(nc) as tc:
        if reps == 1:
            _emit(tc, out, xqt, xkt, xvt, wq, wk, wv, wp, bq, bk, stage=stage)
        else:
            with tc.For_i(0, reps, 1):
                _emit(tc, out, xqt, xkt, xvt, wq, wk, wv, wp, bq, bk,
                      stage=stage)
    nc.compile()
    _built[key] = nc
    return nc


def _in_maps(query, key, value, Wq, bq, Wk, bk, Wv, bv, Wp, bp):
    import ml_dtypes
    b = ml_dtypes.bfloat16
    f = np.float32
    maps = []
    xt = {}
    for n in range(N):
        xt[n] = (
            np.ascontiguousarray(np.asarray(query, f)[n].T.astype(b)),
            np.ascontiguousarray(np.asarray(key, f)[n].T.astype(b)),
            np.ascontiguousarray(np.asarray(value, f)[n].T.astype(b)),
        )
    for c in range(8):
        n, g = divmod(c, 4)
        lo, hi = g * DH, (g + 1) * DH
        maps.append({
            "xqt": xt[n][0],
            "xkt": xt[n][1],
            "xvt": xt[n][2],
            "wq": np.ascontiguousarray(np.asarray(Wq, f)[:, lo:hi].astype(b)),
            "wk": np.ascontiguousarray(np.asarray(Wk, f)[:, lo:hi].astype(b)),
            "wv": np.ascontiguousarray(np.asarray(Wv, f)[:, lo:hi].astype(b)),
            "wp": np.ascontiguousarray(np.asarray(Wp, f)[lo:hi, :].astype(b)),
            "bq": np.ascontiguousarray(np.asarray(bq, f)[lo:hi]),
            "bk": np.ascontiguousarray(np.asarray(bk, f)[lo:hi]),
        })
    return maps


last_results = None


def kernel(query, key, value, Wq, bq, Wk, bk, Wv, bv, Wp, bp, trace=False):
    global last_results
    from concourse import bass_utils

    nc = _build()
    maps = _in_maps(query, key, value, Wq, bq, Wk, bk, Wv, bv, Wp, bp)
    res = bass_utils.run_bass_kernel_spmd(
        nc, maps, core_ids=list(range(8)), trace=trace
    )
    last_results = res

    out = np.empty((N, S, D), np.float32)
    bvp = np.asarray(bv, np.float64) @ np.asarray(Wp, np.float64)
    for n in range(N):
        acc = np.zeros((S, D), np.float64)
        for g in range(4):
            acc += res.results[4 * n + g]["out"].astype(np.float64)
        acc += bvp + np.asarray(bp, np.float64)
        out[n] = acc.astype(np.float32)
    return out


# revision 3
# speedup vs baseline: 1.0815x; 1.0815x over previous
"""TRN2 Bass/Tile kernel v2: 16-head MHA (N=2, S=2048, D=1024) on 8 NeuronCores.

Sharding (hardcoded): core c = 4*n + g runs batch n (data parallel) and head
group g (tensor parallel, 4 heads).  Wq/Wk/Wv column-sharded [1024, 256], Wp
row-sharded [256, 1024].  Host sums the 4 partials per batch and adds
(bv @ Wp + bp).

v2 changes vs baseline:
  - all inputs host-cast to bf16 (matmuls were bf16 anyway): halves DMA-in,
    removes the on-chip fp32->bf16 cast traffic on DVE/ACT entirely
  - scores matmuls row-tiled: the two heads of a pair occupy PE row strips
    0-63 / 64-127 (K=64 contraction each), so their LDWEIGHTS/streams overlap
    (measured 460 -> 201 ns/MM)
  - softmax exp runs on ScalarE at [128, 1024] tiles straight out of PSUM
  - projections are emitted in DMA-lockstep order (Q both blocks while xk
    streams in, K as each xk chunk lands, V as xv lands) so the load phase
    hides all projection compute (proj stage adds 1.5us over the 48us load)
  - output projection is split per query block and interleaved into the next
    block's attention; partial outs are stored bf16
  - measured constraint that caps further overlap: any engine reading PSUM
    (ACT exp, DVE evictions) serializes against concurrent TensorE PSUM
    writes (microbench: scores+exp+AV window = 4364ns ~= PE-only 1591 +
    ACT-only 2133, and exp-from-SBUF restores 2260ns but the eviction that
    feeds it costs the same PSUM-read time on DVE).  The kernel is at
    ~365us against a ~300-330us fully-serial floor (PE stream + one
    PSUM-read traversal of all scores + evictions).
"""

import numpy as np

N, S, D = 2, 2048, 1024
H, HD = 16, 64
NHL = 4                 # heads per core
DH = NHL * HD           # 256 local channels
P = 128
KC = D // P             # 8 contraction chunks for the projections
SC = S // P             # 16 key chunks
IH = 1024               # query block width
NI = S // IH            # 2 query blocks

_built = {}


def _emit(tc, out, xqt, xkt, xvt, wq, wk, wv, wp, bq, bk, stage="full"):
    from concourse import mybir

    nc = tc.nc
    f32 = mybir.dt.float32
    bf16 = mybir.dt.bfloat16
    Exp = mybir.ActivationFunctionType.Exp
    MUL = mybir.AluOpType.mult
    ADD = mybir.AluOpType.add

    with (
        tc.tile_pool(name="const", bufs=1) as cpool,
        tc.tile_pool(name="work", bufs=1) as wpool,
        tc.tile_pool(name="e", bufs=5) as epool,
        tc.tile_pool(name="small", bufs=2) as spool,
        tc.tile_pool(name="ob", bufs=4) as opool,
        tc.tile_pool(name="ps", bufs=1, space="PSUM") as ps,
    ):
        # ---------- SBUF tiles ----------
        wq_sb = cpool.tile([P, KC, DH], bf16)
        wk_sb = cpool.tile([P, KC, DH], bf16)
        wv_sb = cpool.tile([P, KC, DH], bf16)
        wp_sb = cpool.tile([P, 2, D], bf16)
        bq_sb = cpool.tile([P, 2], f32)
        bk_sb = cpool.tile([P, 2], f32)
        xq_sb = wpool.tile([P, KC, S], bf16)
        xk_sb = wpool.tile([P, KC, S], bf16)
        xv_sb = wpool.tile([P, KC, S], bf16)
        qt_sb = wpool.tile([P, 2, S], bf16)
        kt_sb = wpool.tile([P, 2, S], bf16)
        v_sb = wpool.tile([P, SC, NHL, HD + 1], bf16)
        ot_sb = {}  # (pair, i) -> [128, IH] bf16

        # ACT exp-table preload: tiny exp early so the ~2.7us table DMA
        # overlaps the input loads.
        warm = spool.tile([1, 8], f32, tag="warm", name="warm")
        nc.vector.memset(warm[:], 0.0)
        wout = spool.tile([1, 8], bf16, tag="wout", name="wout")
        nc.scalar.activation(wout[:], warm[:], Exp, scale=0.125)

        nc.vector.memset(v_sb[:], 1.0)

        # ---------- DMA issue ----------
        # weights + biases on the SWDGE (gpsimd) queue, activations on the
        # HWDGE (sync) queue; the two rings drain concurrently.
        nc.gpsimd.dma_start(wq_sb[:], wq.rearrange("(kc p) d -> p kc d", p=P))
        nc.gpsimd.dma_start(wk_sb[:], wk.rearrange("(kc p) d -> p kc d", p=P))
        nc.gpsimd.dma_start(bq_sb[:], bq.rearrange("(c p) -> p c", p=P))
        nc.gpsimd.dma_start(bk_sb[:], bk.rearrange("(c p) -> p c", p=P))
        nc.gpsimd.dma_start(wv_sb[:], wv.rearrange("(kc p) d -> p kc d", p=P))
        nc.gpsimd.dma_start(wp_sb[:], wp.rearrange("(c p) e -> p c e", p=P))
        for kc in range(KC):
            nc.sync.dma_start(xq_sb[:, kc, :], xqt[kc * P:(kc + 1) * P, :])
        for kc in range(KC):
            nc.sync.dma_start(xk_sb[:, kc, :], xkt[kc * P:(kc + 1) * P, :])
        for kc in range(KC):
            nc.sync.dma_start(xv_sb[:, kc, :], xvt[kc * P:(kc + 1) * P, :])

        if stage == "load":
            return

        # ---------- Q/K projections ----------
        # Q^T / K^T [256, 2048]: partitions = head-pair dims (pair c on
        # kt_sb[:, c, :], head e of the pair on partitions 64e..64e+63).
        PTAGS = ["sc0", "sc1", "av0", "av1"]

        def qkproj(x_sb, w_sb, b_sb, dst, lo, hi):
            # project x columns [lo, hi) (queries or keys)
            nblk = (hi - lo) // 512
            for c in range(2):
                pts = [
                    ps.tile([P, 512], f32, tag=PTAGS[b % 4], name=f"qk{c}_{b}")
                    for b in range(nblk)
                ]
                for kc in range(KC):
                    for b in range(nblk):
                        nc.tensor.matmul(
                            pts[b][:],
                            lhsT=w_sb[:, kc, c * P:(c + 1) * P],
                            rhs=x_sb[:, kc, lo + b * 512:lo + (b + 1) * 512],
                            start=(kc == 0),
                            stop=(kc == KC - 1),
                        )
                for b in range(nblk):
                    nc.vector.tensor_scalar(
                        dst[:, c, lo + b * 512:lo + (b + 1) * 512],
                        pts[b][:], b_sb[:, c:c + 1], None, ADD,
                    )

        def kproj_all():
            # c-wave of 4 psum groups accumulates as each xk chunk's DMA
            # lands (keeps PE paced with the DMA and HAM warm); c=0 (pair 0)
            # finishes first so attention can start while c=1 drains
            for c in range(2):
                pts = [
                    ps.tile([P, 512], f32, tag=PTAGS[b], name=f"k{c}_{b}")
                    for b in range(4)
                ]
                for kc in range(KC):
                    for b in range(4):
                        nc.tensor.matmul(
                            pts[b][:],
                            lhsT=wk_sb[:, kc, c * P:(c + 1) * P],
                            rhs=xk_sb[:, kc, b * 512:(b + 1) * 512],
                            start=(kc == 0),
                            stop=(kc == KC - 1),
                        )
                    nc.tensor.ldweights(wk_sb[:, kc, 0:P])
                for b in range(4):
                    nc.vector.tensor_scalar(
                        kt_sb[:, c, b * 512:(b + 1) * 512],
                        pts[b][:], bk_sb[:, c:c + 1], None, ADD,
                    )

        qkproj(xq_sb, wq_sb, bq_sb, qt_sb, 0, 1024)      # Q^T block i0
        qkproj(xq_sb, wq_sb, bq_sb, qt_sb, 1024, 2048)   # Q^T block i1
        kproj_all()                                      # K^T all keys

        def vproj(jc):
            pv = ps.tile([P, DH], f32, tag=PTAGS[jc % 4], name="vp")
            for kc in range(KC):
                nc.tensor.matmul(
                    pv[:],
                    lhsT=xv_sb[:, kc, jc * P:(jc + 1) * P],
                    rhs=wv_sb[:, kc, :],
                    start=(kc == 0),
                    stop=(kc == KC - 1),
                )
            nc.vector.tensor_copy(
                v_sb[:, jc, :, 0:HD], pv.rearrange("p (h d) -> p h d", d=HD)
            )

        for jc in range(SC):
            vproj(jc)

        if stage == "proj":
            return

        # ---------- attention + interleaved outproj ----------
        # out partial rows for query block i: psum <- ot_p0 @ wp[p0] + ot_p1
        # @ wp[p1]; evict bf16 and store.  Emitted as a generator so groups
        # can be interleaved into the next block's key-chunk loop.
        def outproj_groups(i):
            for qc in range(IH // P):
                r0 = i * IH + qc * P
                for eh in range(2):
                    def grp(qc=qc, eh=eh, r0=r0):
                        po = ps.tile([P, 512], f32, tag=PTAGS[(qc + eh) % 2],
                                     name="po")
                        for pr in range(2):
                            nc.tensor.matmul(
                                po[:],
                                lhsT=ot_sb[(pr, i)][:, qc * P:(qc + 1) * P],
                                rhs=wp_sb[:, pr, eh * 512:(eh + 1) * 512],
                                start=(pr == 0),
                                stop=(pr == 1),
                            )
                        ob = opool.tile([P, 512], out.dtype, tag="ob", name="ob")
                        nc.vector.tensor_copy(ob[:], po[:])
                        nc.sync.dma_start(
                            out[r0:r0 + P, eh * 512:(eh + 1) * 512], ob[:])
                    yield grp

        pending_out = None  # generator of outproj groups to interleave

        for bi, (p, i) in enumerate([(0, 0), (1, 0), (0, 1), (1, 1)]):
            av = [
                ps.tile([HD + 1, IH], f32, tag=f"av{e}", name=f"av{e}")
                for e in range(2)
            ]
            i0 = i * IH

            def emit_av(jc, ets):
                # A@V one chunk behind the scores, so the in-order PE stream
                # never waits on ScalarE exp
                for e in range(2):
                    for iq in range(2):
                        nc.tensor.matmul(
                            av[e][:, iq * 512:(iq + 1) * 512],
                            lhsT=v_sb[:, jc, 2 * p + e, :],
                            rhs=ets[e][:, iq * 512:(iq + 1) * 512],
                            start=(jc == 0),
                            stop=(jc == SC - 1),
                        )

            pending_av = None
            for jc in range(SC):
                sc = [
                    ps.tile([P, IH], f32, tag=f"sc{e}", name=f"sc{e}")
                    for e in range(2)
                ]
                for iq in range(2):
                    for e in range(2):
                        nc.tensor.matmul(
                            sc[e][:, iq * 512:(iq + 1) * 512],
                            lhsT=kt_sb[HD * e:HD * (e + 1), p,
                                       jc * P:(jc + 1) * P],
                            rhs=qt_sb[HD * e:HD * (e + 1), p,
                                      i0 + iq * 512:i0 + (iq + 1) * 512],
                            start=True,
                            stop=True,
                        )
                ets = []
                for e in range(2):
                    et = epool.tile([P, IH], bf16, tag="e", name="et")
                    nc.scalar.activation(et[:], sc[e][:], Exp, scale=0.125)
                    ets.append(et)
                if pending_av is not None:
                    emit_av(*pending_av)
                    if pending_out is not None:
                        try:
                            next(pending_out)()
                        except StopIteration:
                            pending_out = None
                    else:
                        # HAM keep-alive: pad the ACT-bound window so the PE
                        # activity monitor never sees an idle SHORT window
                        nc.tensor.ldweights(kt_sb[:, p, jc * P:jc * P + P])
                        nc.tensor.ldweights(qt_sb[:, p, i0:i0 + P])
                pending_av = (jc, ets)
            if pending_av is not None:
                emit_av(*pending_av)
            # normalize: O^T = O^T_un * (1/denom); denom is av row 64
            ot = wpool.tile([P, IH], bf16, tag=f"ot{p}_{i}", name=f"ot{p}_{i}")
            ot_sb[(p, i)] = ot
            for e in range(2):
                rec1 = spool.tile([HD + 1, IH], f32, tag="rec1", name="rec1")
                nc.vector.reciprocal(rec1[HD:HD + 1, :], av[e][HD:HD + 1, :])
                rec = spool.tile([HD, IH], f32, tag="rec", name="rec")
                nc.sync.dma_start(
                    rec[:],
                    rec1[HD:HD + 1, None, :].to_broadcast((1, HD, IH)),
                )
                if e == 0:
                    nc.vector.tensor_tensor(
                        ot[0:HD, :], av[e][0:HD, :], rec[:], MUL)
                else:
                    otmp = spool.tile([HD, IH], bf16, tag="otmp", name="otmp")
                    nc.vector.tensor_tensor(otmp[:], av[e][0:HD, :], rec[:], MUL)
                    nc.sync.dma_start(ot[HD:P, :], otmp[:])
            if stage == "attn":
                continue
            if p == 1:  # both pairs of block i done -> queue outproj
                # drain any leftover groups of the previous queue first
                if pending_out is not None:
                    for grp in pending_out:
                        grp()
                pending_out = outproj_groups(i)
        if stage == "attn":
            return
        if pending_out is not None:
            for grp in pending_out:
                grp()


def _build(reps=1, stage="full"):
    key = ("nc", reps, stage)
    if key in _built:
        return _built[key]
    import concourse.tile as tile
    from concourse import bacc, mybir

    f32 = mybir.dt.float32
    bf16 = mybir.dt.bfloat16
    nc = bacc.Bacc(
        "TRN2",
        target_bir_lowering=False,
        debug=False,
        num_devices=8,
    )
    xqt = nc.dram_tensor("xqt", [D, S], bf16, kind="ExternalInput").ap()
    xkt = nc.dram_tensor("xkt", [D, S], bf16, kind="ExternalInput").ap()
    xvt = nc.dram_tensor("xvt", [D, S], bf16, kind="ExternalInput").ap()
    wq = nc.dram_tensor("wq", [D, DH], bf16, kind="ExternalInput").ap()
    wk = nc.dram_tensor("wk", [D, DH], bf16, kind="ExternalInput").ap()
    wv = nc.dram_tensor("wv", [D, DH], bf16, kind="ExternalInput").ap()
    wp = nc.dram_tensor("wp", [DH, D], bf16, kind="ExternalInput").ap()
    bq = nc.dram_tensor("bq", [DH], f32, kind="ExternalInput").ap()
    bk = nc.dram_tensor("bk", [DH], f32, kind="ExternalInput").ap()
    out = nc.dram_tensor("out", [S, D], bf16, kind="ExternalOutput").ap()

    with tile.TileContext(nc) as tc:
        if reps == 1:
            _emit(tc, out, xqt, xkt, xvt, wq, wk, wv, wp, bq, bk, stage=stage)
        else:
            with tc.For_i(0, reps, 1):
                _emit(tc, out, xqt, xkt, xvt, wq, wk, wv, wp, bq, bk,
                      stage=stage)
    nc.compile()
    _built[key] = nc
    return nc


def _in_maps(query, key, value, Wq, bq, Wk, bk, Wv, bv, Wp, bp):
    import ml_dtypes
    b = ml_dtypes.bfloat16
    f = np.float32
    maps = []
    xt = {}
    for n in range(N):
        xt[n] = (
            np.ascontiguousarray(np.asarray(query, f)[n].T.astype(b)),
            np.ascontiguousarray(np.asarray(key, f)[n].T.astype(b)),
            np.ascontiguousarray(np.asarray(value, f)[n].T.astype(b)),
        )
    for c in range(8):
        n, g = divmod(c, 4)
        lo, hi = g * DH, (g + 1) * DH
        maps.append({
            "xqt": xt[n][0],
            "xkt": xt[n][1],
            "xvt": xt[n][2],
            "wq": np.ascontiguousarray(np.asarray(Wq, f)[:, lo:hi].astype(b)),
            "wk": np.ascontiguousarray(np.asarray(Wk, f)[:, lo:hi].astype(b)),
            "wv": np.ascontiguousarray(np.asarray(Wv, f)[:, lo:hi].astype(b)),
            "wp": np.ascontiguousarray(np.asarray(Wp, f)[lo:hi, :].astype(b)),
            "bq": np.ascontiguousarray(np.asarray(bq, f)[lo:hi]),
            "bk": np.ascontiguousarray(np.asarray(bk, f)[lo:hi]),
        })
    return maps


last_results = None


def kernel(query, key, value, Wq, bq, Wk, bk, Wv, bv, Wp, bp, trace=False):
    global last_results
    from concourse import bass_utils

    nc = _build()
    maps = _in_maps(query, key, value, Wq, bq, Wk, bk, Wv, bv, Wp, bp)
    res = bass_utils.run_bass_kernel_spmd(
        nc, maps, core_ids=list(range(8)), trace=trace
    )
    last_results = res

    out = np.empty((N, S, D), np.float32)
    bvp = np.asarray(bv, np.float64) @ np.asarray(Wp, np.float64)
    for n in range(N):
        acc = np.zeros((S, D), np.float64)
        for g in range(4):
            acc += res.results[4 * n + g]["out"].astype(np.float64)
        acc += bvp + np.asarray(bp, np.float64)
        out[n] = acc.astype(np.float32)
    return out
